# revision 57
# baseline (speedup 1.0000x reference)
"""Trainium2 Bass kernel for nn_Attention_30554397344218.

Multi-head attention (B=8, S=1040, D=1024, H=16, hd=64) with 2D vision RoPE
on the 1024 grid tokens after a 16-token puzzle prefix.

Sharding: pure data-parallel - one batch element per NeuronCore (8 cores,
no collectives); weights broadcast; host gathers the 8 outputs.

Per-core design (bf16 TensorEngine, f32 PSUM accumulation, ~332us HW):
  - host prepacks every big input chunk-major as [128, chunks*width] so
    each loads with ONE >=1MB DMA (descriptor-bound small transfers cost
    ~20us extra); only xT+Wq issue at t=0 - Wk/Wv/Wo prefetches are paced
    behind phase-1 scalar copies so they don't steal HBM bandwidth from
    the startup-critical transfers; a 40-matmul scratch warmup covers the
    DMA window and trips the HAM clock gate to 2.4GHz (cold PE = 1.2GHz)
  - QKV: q,k produced in transposed layout (head_dim on partitions, two
    heads per 128-partition chunk; k duplicated with one head zeroed so
    score matmuls contract a full K=128 - partial-K stationaries drop the
    PE to the 1.2GHz slow path, measured: K=64 row-tiled pairs run 2x
    SLOWER, tile_position concurrency does not rescue them); v in natural
    layout as overlapped [v_h | ones] blocks at stride 65 (the ones
    column makes attn@v emit the softmax denominator as a 65th output
    row); RoPE swap via a PE permutation matmul + cos/sin muls on DVE
  - scores transposed (keys on partitions); one 1040-wide exp(S/8) on the
    ScalarEngine per (head, key-tile) straight out of a 3-bank PSUM tile;
    att@V interleaved at j-tile granularity with the next head's score
    matmuls; the 16-key j=8 tail is merged per head-PAIR: a packed
    [128x64] stationary (cols 0:16/32:48 = both heads' tail keys) gives
    one matmul stream + one exp for the pair, with a partition-32-aligned
    replica of the 16-row v tail for the odd head's att@v reads
  - normalization deferred: batched reciprocal_approx_fast over head
    colsums (gathered via 1-row DMAs - engine writes must be 32-aligned
    in partition base, so no direct row writes), bf16 selector matmuls
    broadcast the factors (fp32 matmuls run LOW_HIGH at ~4x cost and
    cool the HAM); reciprocal for heads 0-13 hoisted right after head 13
    completes; endgame emits last-head att@v first, then chunk norms on
    the idle "st" PSUM slots, then output-projection it-tile 0's chunk
    0-6 partial accumulation BEFORE the chunk-7 norm so the last colsum
    DMA+rcp latency hides behind real matmuls instead of head-blocking
    the PE FIFO; output staged bf16 and DMA'd per half-tile
  - engine/bandwidth budget per core: PE ~92%, ScalarE (exp) ~94% busy in
    the 151us attention body - the kernel is pinned on both; scores+attv
    are at 50% MAC utilization by construction (hd=64 on a 128-contraction
    array) and fp8/int8 paths lose too much precision for the 2e-2 gate
"""

import numpy as np
import ml_dtypes

B, S, D, H, HD = 8, 1040, 1024, 16, 64
PFX = 16  # puzzle prefix length
GRID = 32
NCHUNK = 8  # 128-row chunks of the (1024,) head-dim axis
NJT = 9  # 128-row tiles of the 1040 seq axis (last tile = 16 rows)
ICH = [(0, 512), (512, 512), (1024, 16)]  # free-dim chunks of the seq axis
BF16 = ml_dtypes.bfloat16

_compiled = None  # cached (nc, const_in_map)


def _rope_tables():
    half, quarter = HD // 2, HD // 4
    frac = 2.0 * np.arange(quarter, dtype=np.float64) / half
    ts = 10000.0 ** frac
    row = np.arange(GRID, dtype=np.float64)[:, None] / ts[None, :]  # (32, 16)
    row_ang = np.broadcast_to(row[:, None, :], (GRID, GRID, quarter)).reshape(
        GRID * GRID, quarter
    )
    col_ang = np.broadcast_to(row[None, :, :], (GRID, GRID, quarter)).reshape(
        GRID * GRID, quarter
    )
    cos64 = np.concatenate(
        [np.cos(row_ang).T, np.cos(row_ang).T, np.cos(col_ang).T, np.cos(col_ang).T],
        axis=0,
    )  # (64, 1024)
    s64 = np.concatenate(
        [-np.sin(row_ang).T, np.sin(row_ang).T, -np.sin(col_ang).T, np.sin(col_ang).T],
        axis=0,
    )
    cosf = np.ones((HD, S), np.float64)
    sf = np.zeros((HD, S), np.float64)
    cosf[:, PFX:] = cos64
    sf[:, PFX:] = s64
    cos2 = np.concatenate([cosf, cosf], axis=0).astype(BF16)  # (128, 1040)
    s2 = np.concatenate([sf, sf], axis=0).astype(BF16)
    return cos2, s2


def _swap_matrix():
    swp = np.zeros((128, 128), np.float32)
    for i in range(128):
        swp[i, i ^ 16] = 1.0
    return swp.astype(BF16)


def _build_body(nc, tc, tile, mybir, aps):
    from contextlib import ExitStack

    bf = mybir.dt.bfloat16
    f32 = mybir.dt.float32
    Exp = mybir.ActivationFunctionType.Exp
    xT, Wq, Wk, Wv, Wo, COS2, S2, SWP, OUT = (
        aps["xT"], aps["Wq"], aps["Wk"], aps["Wv"], aps["Wo"],
        aps["COS2"], aps["S2"], aps["SWP"], aps["out"],
    )
    TAIL = S - 8 * 128  # 16

    with ExitStack() as ctx:
        # ---- persistent pools
        p_qk = ctx.enter_context(tc.tile_pool(name="qk", bufs=24))
        p_vx = ctx.enter_context(tc.tile_pool(name="vx", bufs=9))
        p_ot = ctx.enter_context(tc.tile_pool(name="ot", bufs=8))
        p_tab = ctx.enter_context(tc.tile_pool(name="tab", bufs=1))
        p_wo = ctx.enter_context(tc.tile_pool(name="wo", bufs=8))

        cos_sb = p_tab.tile([128, S], bf, tag="cos")
        s_sb = p_tab.tile([128, S], bf, tag="sin")
        swp_sb = p_tab.tile([128, 128], bf, tag="swp")
        sel16 = p_tab.tile([16, 1024], bf, tag="sel16")
        sel2b = p_tab.tile([2, 128], bf, tag="sel2b")
        ones64 = p_tab.tile([64, S], f32, tag="ones64")
        nc.gpsimd.memset(ones64, 1.0)

        qr = [p_qk.tile([128, S], bf, tag="qk", name=f"qr{i}") for i in range(NCHUNK)]
        krz = [
            [p_qk.tile([128, S], bf, tag="qk", name=f"krz{i}_{z}") for z in range(2)]
            for i in range(NCHUNK)
        ]
        ktl = [
            p_qk.tile([128, 64], bf, tag="ktl", name=f"ktl{i}", bufs=8)
            for i in range(NCHUNK)
        ]
        vx = [
            p_vx.tile([128, 1104], bf, tag="vx", name=f"vx{i}") for i in range(NJT)
        ]
        otc = [p_ot.tile([128, S], bf, tag="ot", name=f"otc{i}") for i in range(NCHUNK)]
        wo_all = p_wo.tile([128, 8 * D], bf, tag="wo", bufs=1)
        wo_t = [wo_all[:, k * D : (k + 1) * D] for k in range(8)]

        # ================= phase 1: projections + RoPE =================
        with ExitStack() as p1:
            p_x = p1.enter_context(tc.tile_pool(name="x", bufs=8))
            p_w = p1.enter_context(tc.tile_pool(name="w", bufs=8))
            p_tmp = p1.enter_context(tc.tile_pool(name="tmp", bufs=3))
            p_ps1 = p1.enter_context(tc.tile_pool(name="ps1", bufs=6, space="PSUM"))
            p_ps2 = p1.enter_context(tc.tile_pool(name="ps2", bufs=2, space="PSUM"))

            # PE warmup: scratch matmuls so HAM reaches 2.4GHz and the PE
            # isn't idle while the startup-critical DMAs (xT+Wq) land
            wa = p_tmp.tile([128, 512], bf, tag="wa", bufs=1)
            wb = p_tmp.tile([128, 128], bf, tag="wb", bufs=1)
            nc.gpsimd.memset(wa, 0.0)
            nc.gpsimd.memset(wb, 0.0)
            wps = p_ps1.tile([128, 512], f32, tag="mm1", name="warm_ps")
            for _w in range(18):
                nc.tensor.matmul(wps, wb, wa, start=True, stop=True)

            # inputs host-prepacked chunk-major as [128, 8*W] so each loads
            # with one >=1MB DMA (341GB/s vs descriptor-bound small ones);
            # only xT+Wq issue now - wk/wv/wo are paced into the pass loop
            # so they don't steal HBM bandwidth from the startup burst
            xt_all = p_x.tile([128, 8 * S], bf, tag="xt", bufs=1)
            wq_all = p_w.tile([128, 8 * D], bf, tag="wq", bufs=1)
            wk_all = p_w.tile([128, 8 * D], bf, tag="wk", bufs=1)
            wv_all = p_w.tile([128, 8 * D], bf, tag="wv", bufs=1)
            xt = [xt_all[:, k * S : (k + 1) * S] for k in range(8)]
            wq_t = [wq_all[:, k * D : (k + 1) * D] for k in range(8)]
            wk_t = [wk_all[:, k * D : (k + 1) * D] for k in range(8)]
            wv_t = [wv_all[:, k * D : (k + 1) * D] for k in range(8)]
            # Wq is host-packed c-major ((c,k) blocks of 128 cols) and split
            # so the first q-pass can start after xT + just its first column
            # block (0.26MB) instead of the full 2.1MB
            nc.sync.dma_start(out=xt_all, in_=xT[:, :])
            nc.scalar.dma_start(out=wq_all[:, 0:1024], in_=Wq[:, 0:1024])
            nc.scalar.dma_start(out=wq_all[:, 1024 : 8 * D], in_=Wq[:, 1024 : 8 * D])
            nc.gpsimd.dma_start(out=swp_sb, in_=SWP[:, :])
            nc.gpsimd.dma_start(out=cos_sb, in_=COS2[:, :])
            nc.gpsimd.dma_start(out=s_sb, in_=S2[:, :])
            nc.gpsimd.dma_start(out=sel16, in_=aps["SEL"][:, :])
            nc.gpsimd.dma_start(out=sel2b, in_=aps["SEL2"][:, :])
            # preload the exp ACT table so phase 2 doesn't pay the switch
            nc.scalar.activation(
                ones64[0:1, 0:8], ones64[0:1, 0:8], Exp, scale=0.0
            )

            for c in range(NCHUNK):
                nc.gpsimd.memset(krz[c][0][64:128, :], 0.0)
                nc.gpsimd.memset(krz[c][1][0:64, :], 0.0)

            # q/k in transposed layout + RoPE, software-pipelined by one
            # chunk so the swap matmul never blocks the PE on the cast copy
            def emit_mm1(which, w_t, c):
                raw = p_tmp.tile([128, S], bf, tag="raw", name=f"raw_{which}{c}")
                pss = [
                    p_ps1.tile([128, 512], f32, tag="mm1", name=f"mm1_{which}{c}_{i}")
                    for i in range(3)
                ]
                for k in range(8):
                    # Wq is c-major packed ((c*8+k) 128-col blocks); Wk keeps
                    # the k-major layout
                    wsl = (
                        wq_all[:, (c * 8 + k) * 128 : (c * 8 + k + 1) * 128]
                        if which == "q"
                        else w_t[k][:, c * 128 : (c + 1) * 128]
                    )
                    for i, (off, wdt) in enumerate(ICH):
                        nc.tensor.matmul(
                            pss[i][:, :wdt],
                            wsl,
                            xt[k][:, off : off + wdt],
                            start=(k == 0),
                            stop=(k == 7),
                        )
                for i, (off, wdt) in enumerate(ICH):
                    nc.scalar.copy(raw[:, off : off + wdt], pss[i][:, :wdt])
                return raw

            def emit_rope(which, c, raw):
                for off, wdt in ICH:
                    sw = p_ps2.tile([128, 512], f32, tag="swp")
                    nc.tensor.matmul(
                        sw[:, :wdt],
                        swp_sb,
                        raw[:, off : off + wdt],
                        start=True,
                        stop=True,
                    )
                    t2 = p_tmp.tile([128, 512], bf, tag="t2")
                    nc.vector.tensor_mul(
                        t2[:, :wdt], sw[:, :wdt], s_sb[:, off : off + wdt]
                    )
                    t1 = p_tmp.tile([128, 512], bf, tag="t1")
                    nc.vector.tensor_mul(
                        t1[:, :wdt],
                        raw[:, off : off + wdt],
                        cos_sb[:, off : off + wdt],
                    )
                    if which == "q":
                        nc.vector.tensor_add(
                            qr[c][:, off : off + wdt], t1[:, :wdt], t2[:, :wdt]
                        )
                    else:
                        nc.vector.tensor_add(
                            krz[c][0][0:64, off : off + wdt],
                            t1[0:64, :wdt],
                            t2[0:64, :wdt],
                        )
                        nc.vector.tensor_add(
                            krz[c][1][64:128, off : off + wdt],
                            t1[64:128, :wdt],
                            t2[64:128, :wdt],
                        )

            steps = [("q", wq_t, c) for c in range(NCHUNK)] + [
                ("k", wk_t, c) for c in range(NCHUNK)
            ]
            pending = None
            for si, (which, w_t, c) in enumerate(steps):
                raw = emit_mm1(which, w_t, c)
                # paced weight prefetch: issued on the scalar queue behind
                # this pass's PSUM->SBUF copies
                if si == 0:
                    nc.scalar.dma_start(out=wk_all, in_=Wk[:, :])
                elif si == 2:
                    nc.scalar.dma_start(out=wv_all, in_=Wv[:, :])
                elif si == 4:
                    nc.scalar.dma_start(out=wo_all, in_=Wo[:, :])
                if pending is not None:
                    emit_rope(*pending)
                pending = (which, c, raw)
            emit_rope(*pending)

            # v in natural layout, overlapped [v_h | 1] blocks (stride 65)
            for j in range(NJT):
                rows = 128 if j < 8 else TAIL
                vx3 = vx[j][:, :1040].rearrange("p (h d) -> p h d", d=65)
                nc.gpsimd.memset(vx[j][:, 1040:1104], 0.0)
                nc.gpsimd.memset(vx3[:rows, :, 64:65], 1.0)
                psv = [
                    p_ps1.tile([128, 512], f32, tag="mm1", name=f"mm1_v{j}_{i}")
                    for i in range(2)
                ]
                for k in range(8):
                    for ci in range(2):
                        nc.tensor.matmul(
                            psv[ci][:rows, :],
                            xt[k][:, j * 128 : j * 128 + rows],
                            wv_t[k][:, ci * 512 : (ci + 1) * 512],
                            start=(k == 0),
                            stop=(k == 7),
                        )
                for ci in range(2):
                    nc.scalar.copy(
                        vx3[:rows, ci * 8 : (ci + 1) * 8, 0:64],
                        psv[ci][:rows, :].rearrange("p (h d) -> p h d", h=8),
                    )

            # packed key-tail stationaries (cols 0:16 head A, 32:48 head B)
            # and a 32-aligned replica of the 16-row v tail so odd heads'
            # j=8 att@v reads probs/values from partition base 32
            for c in range(NCHUNK):
                nc.gpsimd.memset(ktl[c], 0.0)
                nc.vector.tensor_copy(ktl[c][:, 0:16], krz[c][0][:, 1024:1040])
                nc.vector.tensor_copy(ktl[c][:, 32:48], krz[c][1][:, 1024:1040])
            nc.vector.tensor_copy(vx[8][32 : 32 + TAIL, :], vx[8][0:TAIL, :])

        # ================= phase 2: attention per head =================
        with ExitStack() as p2:
            p_pt = p2.enter_context(tc.tile_pool(name="pt", bufs=26))
            p_sm = p2.enter_context(tc.tile_pool(name="sm", bufs=1))
            p_st = p2.enter_context(tc.tile_pool(name="st", bufs=2, space="PSUM"))
            p_po = p2.enter_context(tc.tile_pool(name="po", bufs=2, space="PSUM"))

            cs_all = p_sm.tile([16, S], f32, tag="cs")
            cs_b = p_sm.tile([2, S], f32, tag="csb")
            pt8_cache = {}

            # engine partition bases must be 32-aligned, so colsum rows are
            # staged at partition 64 and gathered into cs via DMA

            def pbase(h, j):
                # row base of head h's probs at key-tile j (j=8 is merged per
                # pair: even head rows 0:16, odd head rows 32:48)
                return 0 if (j < 8 or h % 2 == 0) else 32

            def attv_step(ph, ppts, pot, j):
                rows = 128 if j < 8 else TAIL
                rb = pbase(ph, j)
                for idx in range(2):
                    off, wdt = ICH[idx]
                    nc.tensor.matmul(
                        pot[idx][:, :wdt],
                        vx[j][rb : rb + rows, ph * 65 : ph * 65 + 128],
                        ppts[j][rb : rb + rows, off : off + wdt],
                        start=(j == 0),
                        stop=(j == NJT - 1),
                    )

            def finish_head(ph, ppts, pot, eng=None):
                eng = eng or nc.vector
                pc, phb = divmod(ph, 2)
                cstage = p_sm.tile(
                    [65, S], f32, tag="cstage", name=f"cst{ph}", bufs=2
                )
                # i0/i1 epilogues free po slots, then the 16-wide tail chunk
                for idx in range(2):
                    off, wdt = ICH[idx]
                    eng.tensor_mul(
                        otc[pc][phb * 64 : phb * 64 + 64, off : off + wdt],
                        pot[idx][0:64, :wdt],
                        ones64[:, off : off + wdt],
                    )
                    eng.tensor_copy(
                        cstage[64:65, off : off + wdt], pot[idx][64:65, :wdt]
                    )
                off, wdt = ICH[2]
                ot2 = p_po.tile([128, 512], f32, tag="ot", name=f"ot{ph}_t")
                for j in range(NJT):
                    rows = 128 if j < 8 else TAIL
                    rb = pbase(ph, j)
                    nc.tensor.matmul(
                        ot2[:, :wdt],
                        vx[j][rb : rb + rows, ph * 65 : ph * 65 + 128],
                        ppts[j][rb : rb + rows, off : off + wdt],
                        start=(j == 0),
                        stop=(j == NJT - 1),
                    )
                eng.tensor_mul(
                    otc[pc][phb * 64 : phb * 64 + 64, off : off + wdt],
                    ot2[0:64, :wdt],
                    ones64[:, off : off + wdt],
                )
                eng.tensor_copy(
                    cstage[64:65, off : off + wdt], ot2[64:65, :wdt]
                )
                eng = nc.sync if ph % 2 == 0 else nc.gpsimd
                if ph < 14:
                    eng.dma_start(out=cs_all[ph : ph + 1, :], in_=cstage[64:65, :])
                else:
                    eng.dma_start(
                        out=cs_b[ph - 14 : ph - 13, :], in_=cstage[64:65, :]
                    )

            prev = None
            for h in range(H):
                c, hb = divmod(h, 2)
                if prev is not None:
                    ph, ppts = prev
                    pot = [
                        p_po.tile([128, 512], f32, tag="ot", name=f"ot{ph}_{i}")
                        for i in range(2)
                    ]
                pts = []
                for j in range(NJT):
                    if j < 8:
                        pt = p_pt.tile([128, S], bf, tag="pt", name=f"pt{h}_{j}")
                        pts.append(pt)
                        st = p_st.tile([128, S], f32, tag="st", name=f"st{h}_{j}")
                        for off, wdt in ICH:
                            nc.tensor.matmul(
                                st[:, off : off + wdt],
                                krz[c][hb][:, j * 128 : (j + 1) * 128],
                                qr[c][:, off : off + wdt],
                                start=True,
                                stop=True,
                            )
                        nc.scalar.activation(
                            pt, st, Exp, scale=1.0 / np.sqrt(HD)
                        )
                    elif hb == 0:
                        # merged key-tail for the pair: ktl packs both heads'
                        # 16 tail keys (cols 0:16 / 32:48, rest zero) so one
                        # matmul+exp serves heads 2c and 2c+1
                        pt8 = p_pt.tile([128, S], bf, tag="pt", name=f"pt8_{c}")
                        st8 = p_st.tile([128, S], f32, tag="st", name=f"st8_{c}")
                        for off, wdt in ICH:
                            nc.tensor.matmul(
                                st8[0:64, off : off + wdt],
                                ktl[c],
                                qr[c][:, off : off + wdt],
                                start=True,
                                stop=True,
                            )
                        nc.scalar.activation(
                            pt8[0:48, :], st8[0:48, :], Exp,
                            scale=1.0 / np.sqrt(HD),
                        )
                        pt8_cache[c] = pt8
                        pts.append(pt8)
                    else:
                        pts.append(pt8_cache[c])
                    if prev is not None:
                        attv_step(ph, ppts, pot, j)
                if prev is not None:
                    finish_head(ph, ppts, pot)
                    if ph == 13:
                        # heads 0-13 colsums complete: compute the batched
                        # reciprocals now so the endgame norm matmuls start
                        # without any DVE latency in front of them
                        rcp_a = p_sm.tile([14, S], f32, tag="rcpa")
                        nc.vector.reciprocal_approx_fast(rcp_a, cs_all[0:14, :])
                        rcp_ab = p_sm.tile([14, S], bf, tag="rcpab")
                        nc.vector.tensor_copy(rcp_ab, rcp_a)
                prev = (h, pts)

            # normalize chunks 0-6 first (rcp was hoisted to head 13, so the
            # psb matmuls start immediately); the DVE mul chain then drains
            # under the last head's att@v matmuls, and every otc chunk is
            # ready before the output projection reads it
            for c in range(7):
                psb = p_st.tile([128, S], f32, tag="st", name=f"psb{c}")
                for off, wdt in ICH:
                    nc.tensor.matmul(
                        psb[:, off : off + wdt],
                        sel16[0:14, c * 128 : (c + 1) * 128],
                        rcp_ab[:, off : off + wdt],
                        start=True,
                        stop=True,
                    )
                    nc.vector.tensor_mul(
                        otc[c][:, off : off + wdt],
                        otc[c][:, off : off + wdt],
                        psb[:, off : off + wdt],
                    )

            ph, ppts = prev
            pot = [
                p_po.tile([128, 512], f32, tag="ot", name=f"ot{ph}_{i}")
                for i in range(2)
            ]
            for j in range(NJT):
                attv_step(ph, ppts, pot, j)
            finish_head(ph, ppts, pot)

            rcp_b = p_sm.tile([2, S], f32, tag="rcpb")
            nc.vector.reciprocal_approx_fast(rcp_b, cs_b)
            rcp_bb = p_sm.tile([2, S], bf, tag="rcpbb")
            nc.vector.tensor_copy(rcp_bb, rcp_b)

            # ---- output projection (same psum scope: no pool barrier).
            # it-tile 0's chunk 0-6 accumulation is emitted BEFORE chunk 7's
            # normalization so the cs_b->rcp_b wait hides behind real work
            # instead of head-blocking the PE queue.
            p_y = p2.enter_context(tc.tile_pool(name="y", bufs=4))

            def outproj_tile(it, yps, crange, start_c, stop_c):
                rows = 128 if it < 8 else TAIL
                for c in crange:
                    for ci in range(2):
                        nc.tensor.matmul(
                            yps[ci][:rows, :],
                            otc[c][:, it * 128 : it * 128 + rows],
                            wo_t[c][:, ci * 512 : (ci + 1) * 512],
                            start=(c == start_c),
                            stop=(c == stop_c),
                        )

            def outproj_drain(it, yps):
                rows = 128 if it < 8 else TAIL
                for ci in range(2):
                    ysb = p_y.tile([128, 512], bf, tag="ysb")
                    nc.scalar.copy(ysb[:rows, :], yps[ci][:rows, :])
                    (nc.sync if ci == 0 else nc.gpsimd).dma_start(
                        out=OUT[0:rows, it * D + ci * 512 : it * D + (ci + 1) * 512],
                        in_=ysb[:rows, :],
                    )

            yps0 = [
                p_st.tile([128, 512], f32, tag="st", name="y0_0"),
                p_po.tile([128, 512], f32, tag="ot", name="y0_1"),
            ]
            outproj_tile(0, yps0, range(7), 0, 7)

            psb7 = p_st.tile([128, S], f32, tag="st", name="psb7")
            for oi, (off, wdt) in enumerate(ICH):
                nc.tensor.matmul(
                    psb7[:, off : off + wdt],
                    sel2b,
                    rcp_bb[:, off : off + wdt],
                    start=True,
                    stop=True,
                )
                nc.vector.tensor_mul(
                    otc[7][:, off : off + wdt],
                    otc[7][:, off : off + wdt],
                    psb7[:, off : off + wdt],
                )

            outproj_tile(0, yps0, [7], 0, 7)
            outproj_drain(0, yps0)
            for it in range(1, NJT):
                yps = [
                    p_st.tile([128, 512], f32, tag="st", name=f"y{it}_0"),
                    p_po.tile([128, 512], f32, tag="ot", name=f"y{it}_1"),
                ]
                outproj_tile(it, yps, range(NCHUNK), 0, 7)
                outproj_drain(it, yps)


def _build():
    global _compiled
    if _compiled is not None:
        return _compiled
    import concourse.bass as bass  # noqa: F401
    import concourse.mybir as mybir
    import concourse.tile as tile
    from concourse import bacc

    nc = bacc.Bacc("TRN2", target_bir_lowering=False, debug=False)
    bf = mybir.dt.bfloat16
    f32 = mybir.dt.float32
    # big inputs host-prepacked chunk-major as [128, chunks*width]
    aps = {
        "xT": nc.dram_tensor("xT", [128, 8 * S], bf, kind="ExternalInput").ap(),
        "Wq": nc.dram_tensor("Wq", [128, 8 * D], bf, kind="ExternalInput").ap(),
        "Wk": nc.dram_tensor("Wk", [128, 8 * D], bf, kind="ExternalInput").ap(),
        "Wv": nc.dram_tensor("Wv", [128, 8 * D], bf, kind="ExternalInput").ap(),
        "Wo": nc.dram_tensor("Wo", [128, 8 * D], bf, kind="ExternalInput").ap(),
        "COS2": nc.dram_tensor("COS2", [128, S], bf, kind="ExternalInput").ap(),
        "S2": nc.dram_tensor("S2", [128, S], bf, kind="ExternalInput").ap(),
        "SWP": nc.dram_tensor("SWP", [128, 128], bf, kind="ExternalInput").ap(),
        "SEL": nc.dram_tensor("SEL", [16, 1024], bf, kind="ExternalInput").ap(),
        "SEL2": nc.dram_tensor("SEL2", [2, 128], bf, kind="ExternalInput").ap(),
        "out": nc.dram_tensor("out", [128, 9 * D], bf, kind="ExternalOutput").ap(),
    }
    with tile.TileContext(nc) as tc:
        _build_body(nc, tc, tile, mybir, aps)
    nc.compile()
    _compiled = nc
    return nc


def _install_trace_shim():
    """The agent image's antenv lacks axon_hooks, so run_bass_kernel_spmd's
    trace path can't find the NTFF profile hook trn_boot would have set.
    Recreate the module and install the ctypes hook; skip the S3 artifact
    upload (no creds needed for local timing)."""
    import sys
    import types

    if "antenv.axon_hooks" not in sys.modules:
        import antenv  # noqa: F401

        mod = types.ModuleType("antenv.axon_hooks")
        mod._hook = None

        def set_axon_ntff_profile_hook(h):
            mod._hook = h

        def get_axon_ntff_profile_hook():
            return mod._hook

        mod.set_axon_ntff_profile_hook = set_axon_ntff_profile_hook
        mod.get_axon_ntff_profile_hook = get_axon_ntff_profile_hook
        sys.modules["antenv.axon_hooks"] = mod

    import antenv.axon_hooks as ah

    if ah.get_axon_ntff_profile_hook() is None:
        from trn_agent_boot.trn_boot import _ntff_profile_via_ctypes

        ah.set_axon_ntff_profile_hook(
            _ntff_profile_via_ctypes("/opt/axon/libaxon_pjrt.so")
        )

    import concourse.bass_utils as bu

    bu.upload_artifacts = lambda tmpdir: f"local://{tmpdir}"


def run(inputs, trace=False):
    """Returns (output (8,1040,1024) f32, exec_time_ns or None)."""
    if trace:
        _install_trace_shim()
    from concourse.bass_utils import run_bass_kernel_spmd

    nc = _build()

    def chunk_major(a):  # [8*128, W] -> [128, 8*W]
        w = a.shape[1]
        return np.ascontiguousarray(
            a.reshape(8, 128, w).transpose(1, 0, 2).reshape(128, 8 * w)
        )

    x = np.asarray(inputs["x"], np.float32)
    # Wq packed c-major: block (c*8+k) holds Wq[k*128:(k+1)*128, c*128:(c+1)*128]
    wq = np.ascontiguousarray(
        np.asarray(inputs["Wq"], np.float32)
        .reshape(8, 128, 8, 128)
        .transpose(1, 2, 0, 3)
        .reshape(128, 8 * D)
    ).astype(BF16)
    wk = chunk_major(np.asarray(inputs["Wk"], np.float32).reshape(D, H * HD)).astype(BF16)
    wv = chunk_major(np.asarray(inputs["Wv"], np.float32).reshape(D, H * HD)).astype(BF16)
    wo = chunk_major(np.asarray(inputs["Wo"], np.float32).reshape(H * HD, D)).astype(BF16)
    cos2, s2 = _rope_tables()
    swp = _swap_matrix()
    sel = np.zeros((16, 1024), np.float32)
    for c in range(8):
        for hb in range(2):
            sel[2 * c + hb, c * 128 + hb * 64 : c * 128 + hb * 64 + 64] = 1.0
    sel2b = np.zeros((2, 128), np.float32)
    sel2b[0, 0:64] = 1.0
    sel2b[1, 64:128] = 1.0
    shared = {
        "Wq": wq, "Wk": wk, "Wv": wv, "Wo": wo,
        "COS2": cos2, "S2": s2, "SWP": swp,
        "SEL": sel.astype(BF16), "SEL2": sel2b.astype(BF16),
    }
    in_maps = [
        dict(shared, xT=chunk_major(np.ascontiguousarray(x[b].T)).astype(BF16))
        for b in range(B)
    ]
    res = run_bass_kernel_spmd(nc, in_maps, core_ids=list(range(B)), trace=trace)
    out = np.stack(
        [
            np.asarray(r["out"], np.float32)
            .reshape(128, 9, D)
            .transpose(1, 0, 2)
            .reshape(9 * 128, D)[:S]
            for r in res.results
        ],
        axis=0,
    )
    return out, res.exec_time_ns


def kernel(x, Wq, Wk, Wv, Wo):
    out, _ = run({"x": x, "Wq": Wq, "Wk": Wk, "Wv": Wv, "Wo": Wo})
    return out



# revision 58
# speedup vs baseline: 1.1795x; 1.1795x over previous
"""Trainium2 Bass kernel for nn_Attention_30554397344218.

Multi-head attention (B=8, S=1040, D=1024, H=16, hd=64) with 2D vision RoPE
on the 1024 grid tokens after a 16-token puzzle prefix.

Sharding: pure data-parallel - one batch element per NeuronCore (8 cores,
no collectives); weights broadcast; host gathers the 8 outputs.

Per-core design (bf16 TensorEngine, f32 PSUM accumulation, ~332us HW):
  - host prepacks every big input chunk-major as [128, chunks*width] so
    each loads with ONE >=1MB DMA (descriptor-bound small transfers cost
    ~20us extra); only xT+Wq issue at t=0 - Wk/Wv/Wo prefetches are paced
    behind phase-1 scalar copies so they don't steal HBM bandwidth from
    the startup-critical transfers; a 40-matmul scratch warmup covers the
    DMA window and trips the HAM clock gate to 2.4GHz (cold PE = 1.2GHz)
  - QKV: q,k produced in transposed layout (head_dim on partitions, two
    heads per 128-partition chunk; k duplicated with one head zeroed so
    score matmuls contract a full K=128 - partial-K stationaries drop the
    PE to the 1.2GHz slow path, measured: K=64 row-tiled pairs run 2x
    SLOWER, tile_position concurrency does not rescue them); v in natural
    layout as overlapped [v_h | ones] blocks at stride 65 (the ones
    column makes attn@v emit the softmax denominator as a 65th output
    row); RoPE swap via a PE permutation matmul + cos/sin muls on DVE
  - scores transposed (keys on partitions); one 1040-wide exp(S/8) on the
    ScalarEngine per (head, key-tile) straight out of a 3-bank PSUM tile;
    att@V interleaved at j-tile granularity with the next head's score
    matmuls; the 16-key j=8 tail is merged per head-PAIR: a packed
    [128x64] stationary (cols 0:16/32:48 = both heads' tail keys) gives
    one matmul stream + one exp for the pair, with a partition-32-aligned
    replica of the 16-row v tail for the odd head's att@v reads
  - normalization deferred: batched reciprocal_approx_fast over head
    colsums (gathered via 1-row DMAs - engine writes must be 32-aligned
    in partition base, so no direct row writes), bf16 selector matmuls
    broadcast the factors (fp32 matmuls run LOW_HIGH at ~4x cost and
    cool the HAM); reciprocal for heads 0-13 hoisted right after head 13
    completes; endgame emits last-head att@v first, then chunk norms on
    the idle "st" PSUM slots, then output-projection it-tile 0's chunk
    0-6 partial accumulation BEFORE the chunk-7 norm so the last colsum
    DMA+rcp latency hides behind real matmuls instead of head-blocking
    the PE FIFO; output staged bf16 and DMA'd per half-tile
  - engine/bandwidth budget per core: PE ~92%, ScalarE (exp) ~94% busy in
    the 151us attention body - the kernel is pinned on both; scores+attv
    are at 50% MAC utilization by construction (hd=64 on a 128-contraction
    array) and fp8/int8 paths lose too much precision for the 2e-2 gate
"""

import numpy as np
import ml_dtypes

B, S, D, H, HD = 8, 1040, 1024, 16, 64
PFX = 16  # puzzle prefix length
GRID = 32
NCHUNK = 8  # 128-row chunks of the (1024,) head-dim axis
NJT = 9  # 128-row tiles of the 1040 seq axis (last tile = 16 rows)
ICH = [(0, 512), (512, 512), (1024, 16)]  # free-dim chunks of the seq axis
BF16 = ml_dtypes.bfloat16

_compiled = None  # cached (nc, const_in_map)


def _rope_tables():
    half, quarter = HD // 2, HD // 4
    frac = 2.0 * np.arange(quarter, dtype=np.float64) / half
    ts = 10000.0 ** frac
    row = np.arange(GRID, dtype=np.float64)[:, None] / ts[None, :]  # (32, 16)
    row_ang = np.broadcast_to(row[:, None, :], (GRID, GRID, quarter)).reshape(
        GRID * GRID, quarter
    )
    col_ang = np.broadcast_to(row[None, :, :], (GRID, GRID, quarter)).reshape(
        GRID * GRID, quarter
    )
    cos64 = np.concatenate(
        [np.cos(row_ang).T, np.cos(row_ang).T, np.cos(col_ang).T, np.cos(col_ang).T],
        axis=0,
    )  # (64, 1024)
    s64 = np.concatenate(
        [-np.sin(row_ang).T, np.sin(row_ang).T, -np.sin(col_ang).T, np.sin(col_ang).T],
        axis=0,
    )
    cosf = np.ones((HD, S), np.float64)
    sf = np.zeros((HD, S), np.float64)
    cosf[:, PFX:] = cos64
    sf[:, PFX:] = s64
    cos2 = np.concatenate([cosf, cosf], axis=0).astype(BF16)  # (128, 1040)
    s2 = np.concatenate([sf, sf], axis=0).astype(BF16)
    return cos2, s2


def _swap_matrix():
    swp = np.zeros((128, 128), np.float32)
    for i in range(128):
        swp[i, i ^ 16] = 1.0
    return swp.astype(BF16)


def _build_body(nc, tc, tile, mybir, aps):
    from contextlib import ExitStack

    bf = mybir.dt.bfloat16
    f32 = mybir.dt.float32
    Exp = mybir.ActivationFunctionType.Exp
    xT, Wq, Wk, Wv, Wo, COS2, S2, SWP, OUT = (
        aps["xT"], aps["Wq"], aps["Wk"], aps["Wv"], aps["Wo"],
        aps["COS2"], aps["S2"], aps["SWP"], aps["out"],
    )
    TAIL = S - 8 * 128  # 16

    with ExitStack() as ctx:
        # ---- persistent pools
        p_qk = ctx.enter_context(tc.tile_pool(name="qk", bufs=24))
        p_vx = ctx.enter_context(tc.tile_pool(name="vx", bufs=9))
        p_ot = ctx.enter_context(tc.tile_pool(name="ot", bufs=8))
        p_tab = ctx.enter_context(tc.tile_pool(name="tab", bufs=1))
        p_wo = ctx.enter_context(tc.tile_pool(name="wo", bufs=8))

        cos_sb = p_tab.tile([128, S], bf, tag="cos")
        s_sb = p_tab.tile([128, S], bf, tag="sin")
        swp_sb = p_tab.tile([128, 128], bf, tag="swp")
        sel16 = p_tab.tile([16, 1024], bf, tag="sel16")
        sel2b = p_tab.tile([2, 128], bf, tag="sel2b")
        ones64 = p_tab.tile([64, S], f32, tag="ones64")
        nc.gpsimd.memset(ones64, 1.0)

        qr = [p_qk.tile([128, S], bf, tag="qk", name=f"qr{i}") for i in range(NCHUNK)]
        krz = [
            [p_qk.tile([128, S], bf, tag="qk", name=f"krz{i}_{z}") for z in range(2)]
            for i in range(NCHUNK)
        ]
        ktl = [
            p_qk.tile([128, 64], bf, tag="ktl", name=f"ktl{i}", bufs=8)
            for i in range(NCHUNK)
        ]
        vx = [
            p_vx.tile([128, 1104], bf, tag="vx", name=f"vx{i}") for i in range(NJT)
        ]
        otc = [p_ot.tile([128, S], bf, tag="ot", name=f"otc{i}") for i in range(NCHUNK)]
        wo_all = p_wo.tile([128, 8 * D], bf, tag="wo", bufs=1)
        wo_t = [wo_all[:, k * D : (k + 1) * D] for k in range(8)]

        # ================= phase 1: projections + RoPE =================
        with ExitStack() as p1:
            p_x = p1.enter_context(tc.tile_pool(name="x", bufs=8))
            p_w = p1.enter_context(tc.tile_pool(name="w", bufs=8))
            p_tmp = p1.enter_context(tc.tile_pool(name="tmp", bufs=3))
            p_ps1 = p1.enter_context(tc.tile_pool(name="ps1", bufs=6, space="PSUM"))
            p_ps2 = p1.enter_context(tc.tile_pool(name="ps2", bufs=2, space="PSUM"))

            # PE warmup: scratch matmuls so HAM reaches 2.4GHz and the PE
            # isn't idle while the startup-critical DMAs (xT+Wq) land
            wa = p_tmp.tile([128, 512], bf, tag="wa", bufs=1)
            wb = p_tmp.tile([128, 128], bf, tag="wb", bufs=1)
            nc.gpsimd.memset(wa, 0.0)
            nc.gpsimd.memset(wb, 0.0)
            wps = p_ps1.tile([128, 512], f32, tag="mm1", name="warm_ps")
            for _w in range(40):
                nc.tensor.matmul(wps, wb, wa, start=True, stop=True)

            # inputs host-prepacked chunk-major as [128, 8*W] so each loads
            # with one >=1MB DMA (341GB/s vs descriptor-bound small ones);
            # only xT+Wq issue now - wk/wv/wo are paced into the pass loop
            # so they don't steal HBM bandwidth from the startup burst
            xt_all = p_x.tile([128, 8 * S], bf, tag="xt", bufs=1)
            wq_all = p_w.tile([128, 8 * D], bf, tag="wq", bufs=1)
            wk_all = p_w.tile([128, 8 * D], bf, tag="wk", bufs=1)
            wv_all = p_w.tile([128, 8 * D], bf, tag="wv", bufs=1)
            xt = [xt_all[:, k * S : (k + 1) * S] for k in range(8)]
            wq_t = [wq_all[:, k * D : (k + 1) * D] for k in range(8)]
            wk_t = [wk_all[:, k * D : (k + 1) * D] for k in range(8)]
            wv_t = [wv_all[:, k * D : (k + 1) * D] for k in range(8)]
            nc.sync.dma_start(out=xt_all, in_=xT[:, :])
            nc.scalar.dma_start(out=wq_all, in_=Wq[:, :])
            nc.gpsimd.dma_start(out=swp_sb, in_=SWP[:, :])
            nc.gpsimd.dma_start(out=cos_sb, in_=COS2[:, :])
            nc.gpsimd.dma_start(out=s_sb, in_=S2[:, :])
            nc.gpsimd.dma_start(out=sel16, in_=aps["SEL"][:, :])
            nc.gpsimd.dma_start(out=sel2b, in_=aps["SEL2"][:, :])
            # preload the exp ACT table so phase 2 doesn't pay the switch
            nc.scalar.activation(
                ones64[0:1, 0:8], ones64[0:1, 0:8], Exp, scale=0.0
            )

            for c in range(NCHUNK):
                nc.gpsimd.memset(krz[c][0][64:128, :], 0.0)
                nc.gpsimd.memset(krz[c][1][0:64, :], 0.0)

            # q/k in transposed layout + RoPE, software-pipelined by one
            # chunk so the swap matmul never blocks the PE on the cast copy
            def emit_mm1(which, w_t, c):
                raw = p_tmp.tile([128, S], bf, tag="raw", name=f"raw_{which}{c}")
                pss = [
                    p_ps1.tile([128, 512], f32, tag="mm1", name=f"mm1_{which}{c}_{i}")
                    for i in range(3)
                ]
                for k in range(8):
                    for i, (off, wdt) in enumerate(ICH):
                        nc.tensor.matmul(
                            pss[i][:, :wdt],
                            w_t[k][:, c * 128 : (c + 1) * 128],
                            xt[k][:, off : off + wdt],
                            start=(k == 0),
                            stop=(k == 7),
                        )
                for i, (off, wdt) in enumerate(ICH):
                    nc.scalar.copy(raw[:, off : off + wdt], pss[i][:, :wdt])
                return raw

            def emit_rope(which, c, raw):
                for off, wdt in ICH:
                    sw = p_ps2.tile([128, 512], f32, tag="swp")
                    nc.tensor.matmul(
                        sw[:, :wdt],
                        swp_sb,
                        raw[:, off : off + wdt],
                        start=True,
                        stop=True,
                    )
                    t2 = p_tmp.tile([128, 512], bf, tag="t2")
                    nc.vector.tensor_mul(
                        t2[:, :wdt], sw[:, :wdt], s_sb[:, off : off + wdt]
                    )
                    t1 = p_tmp.tile([128, 512], bf, tag="t1")
                    nc.vector.tensor_mul(
                        t1[:, :wdt],
                        raw[:, off : off + wdt],
                        cos_sb[:, off : off + wdt],
                    )
                    if which == "q":
                        nc.vector.tensor_add(
                            qr[c][:, off : off + wdt], t1[:, :wdt], t2[:, :wdt]
                        )
                    else:
                        nc.vector.tensor_add(
                            krz[c][0][0:64, off : off + wdt],
                            t1[0:64, :wdt],
                            t2[0:64, :wdt],
                        )
                        nc.vector.tensor_add(
                            krz[c][1][64:128, off : off + wdt],
                            t1[64:128, :wdt],
                            t2[64:128, :wdt],
                        )

            steps = [("q", wq_t, c) for c in range(NCHUNK)] + [
                ("k", wk_t, c) for c in range(NCHUNK)
            ]
            pending = None
            for si, (which, w_t, c) in enumerate(steps):
                raw = emit_mm1(which, w_t, c)
                # paced weight prefetch: issued on the scalar queue behind
                # this pass's PSUM->SBUF copies
                if si == 0:
                    nc.scalar.dma_start(out=wk_all, in_=Wk[:, :])
                elif si == 2:
                    nc.scalar.dma_start(out=wv_all, in_=Wv[:, :])
                elif si == 4:
                    nc.scalar.dma_start(out=wo_all, in_=Wo[:, :])
                if pending is not None:
                    emit_rope(*pending)
                pending = (which, c, raw)
            emit_rope(*pending)

            # v in natural layout, overlapped [v_h | 1] blocks (stride 65)
            for j in range(NJT):
                rows = 128 if j < 8 else TAIL
                vx3 = vx[j][:, :1040].rearrange("p (h d) -> p h d", d=65)
                nc.gpsimd.memset(vx[j][:, 1040:1104], 0.0)
                nc.gpsimd.memset(vx3[:rows, :, 64:65], 1.0)
                psv = [
                    p_ps1.tile([128, 512], f32, tag="mm1", name=f"mm1_v{j}_{i}")
                    for i in range(2)
                ]
                for k in range(8):
                    for ci in range(2):
                        nc.tensor.matmul(
                            psv[ci][:rows, :],
                            xt[k][:, j * 128 : j * 128 + rows],
                            wv_t[k][:, ci * 512 : (ci + 1) * 512],
                            start=(k == 0),
                            stop=(k == 7),
                        )
                for ci in range(2):
                    nc.scalar.copy(
                        vx3[:rows, ci * 8 : (ci + 1) * 8, 0:64],
                        psv[ci][:rows, :].rearrange("p (h d) -> p h d", h=8),
                    )

            # packed key-tail stationaries (cols 0:16 head A, 32:48 head B)
            # and a 32-aligned replica of the 16-row v tail so odd heads'
            # j=8 att@v reads probs/values from partition base 32
            for c in range(NCHUNK):
                nc.gpsimd.memset(ktl[c], 0.0)
                nc.vector.tensor_copy(ktl[c][:, 0:16], krz[c][0][:, 1024:1040])
                nc.vector.tensor_copy(ktl[c][:, 32:48], krz[c][1][:, 1024:1040])
            nc.vector.tensor_copy(vx[8][32 : 32 + TAIL, :], vx[8][0:TAIL, :])

        # ================= phase 2: attention per head =================
        with ExitStack() as p2:
            p_pt = p2.enter_context(tc.tile_pool(name="pt", bufs=26))
            p_sm = p2.enter_context(tc.tile_pool(name="sm", bufs=1))
            p_st = p2.enter_context(tc.tile_pool(name="st", bufs=2, space="PSUM"))
            p_po = p2.enter_context(tc.tile_pool(name="po", bufs=2, space="PSUM"))

            cs_all = p_sm.tile([16, S], f32, tag="cs")
            cs_b = p_sm.tile([2, S], f32, tag="csb")
            pt8_cache = {}

            # engine partition bases must be 32-aligned, so colsum rows are
            # staged at partition 64 and gathered into cs via DMA

            def pbase(h, j):
                # row base of head h's probs at key-tile j (j=8 is merged per
                # pair: even head rows 0:16, odd head rows 32:48)
                return 0 if (j < 8 or h % 2 == 0) else 32

            def attv_step(ph, ppts, pot, j):
                rows = 128 if j < 8 else TAIL
                rb = pbase(ph, j)
                for idx in range(2):
                    off, wdt = ICH[idx]
                    nc.tensor.matmul(
                        pot[idx][:, :wdt],
                        vx[j][rb : rb + rows, ph * 65 : ph * 65 + 128],
                        ppts[j][rb : rb + rows, off : off + wdt],
                        start=(j == 0),
                        stop=(j == NJT - 1),
                    )

            def finish_head(ph, ppts, pot, eng=None):
                eng = eng or nc.vector
                pc, phb = divmod(ph, 2)
                cstage = p_sm.tile(
                    [65, S], f32, tag="cstage", name=f"cst{ph}", bufs=2
                )
                # i0/i1 epilogues free po slots, then the 16-wide tail chunk
                for idx in range(2):
                    off, wdt = ICH[idx]
                    eng.tensor_mul(
                        otc[pc][phb * 64 : phb * 64 + 64, off : off + wdt],
                        pot[idx][0:64, :wdt],
                        ones64[:, off : off + wdt],
                    )
                    eng.tensor_copy(
                        cstage[64:65, off : off + wdt], pot[idx][64:65, :wdt]
                    )
                off, wdt = ICH[2]
                ot2 = p_po.tile([128, 512], f32, tag="ot", name=f"ot{ph}_t")
                for j in range(NJT):
                    rows = 128 if j < 8 else TAIL
                    rb = pbase(ph, j)
                    nc.tensor.matmul(
                        ot2[:, :wdt],
                        vx[j][rb : rb + rows, ph * 65 : ph * 65 + 128],
                        ppts[j][rb : rb + rows, off : off + wdt],
                        start=(j == 0),
                        stop=(j == NJT - 1),
                    )
                eng.tensor_mul(
                    otc[pc][phb * 64 : phb * 64 + 64, off : off + wdt],
                    ot2[0:64, :wdt],
                    ones64[:, off : off + wdt],
                )
                eng.tensor_copy(
                    cstage[64:65, off : off + wdt], ot2[64:65, :wdt]
                )
                eng = nc.sync if ph % 2 == 0 else nc.gpsimd
                if ph < 14:
                    eng.dma_start(out=cs_all[ph : ph + 1, :], in_=cstage[64:65, :])
                else:
                    eng.dma_start(
                        out=cs_b[ph - 14 : ph - 13, :], in_=cstage[64:65, :]
                    )

            prev = None
            for h in range(H):
                c, hb = divmod(h, 2)
                if prev is not None:
                    ph, ppts = prev
                    pot = [
                        p_po.tile([128, 512], f32, tag="ot", name=f"ot{ph}_{i}")
                        for i in range(2)
                    ]
                pts = []
                for j in range(NJT):
                    if j < 8:
                        pt = p_pt.tile([128, S], bf, tag="pt", name=f"pt{h}_{j}")
                        pts.append(pt)
                        st = p_st.tile([128, S], f32, tag="st", name=f"st{h}_{j}")
                        for off, wdt in ICH:
                            nc.tensor.matmul(
                                st[:, off : off + wdt],
                                krz[c][hb][:, j * 128 : (j + 1) * 128],
                                qr[c][:, off : off + wdt],
                                start=True,
                                stop=True,
                            )
                        nc.scalar.activation(
                            pt, st, Exp, scale=1.0 / np.sqrt(HD)
                        )
                    elif hb == 0:
                        # merged key-tail for the pair: ktl packs both heads'
                        # 16 tail keys (cols 0:16 / 32:48, rest zero) so one
                        # matmul+exp serves heads 2c and 2c+1
                        pt8 = p_pt.tile([128, S], bf, tag="pt", name=f"pt8_{c}")
                        st8 = p_st.tile([128, S], f32, tag="st", name=f"st8_{c}")
                        for off, wdt in ICH:
                            nc.tensor.matmul(
                                st8[0:64, off : off + wdt],
                                ktl[c],
                                qr[c][:, off : off + wdt],
                                start=True,
                                stop=True,
                            )
                        nc.scalar.activation(
                            pt8[0:48, :], st8[0:48, :], Exp,
                            scale=1.0 / np.sqrt(HD),
                        )
                        pt8_cache[c] = pt8
                        pts.append(pt8)
                    else:
                        pts.append(pt8_cache[c])
                    if prev is not None:
                        attv_step(ph, ppts, pot, j)
                if prev is not None:
                    finish_head(ph, ppts, pot)
                    if ph == 13:
                        # heads 0-13 colsums complete: compute the batched
                        # reciprocals now so the endgame norm matmuls start
                        # without any DVE latency in front of them
                        rcp_a = p_sm.tile([14, S], f32, tag="rcpa")
                        nc.vector.reciprocal_approx_fast(rcp_a, cs_all[0:14, :])
                        rcp_ab = p_sm.tile([14, S], bf, tag="rcpab")
                        nc.vector.tensor_copy(rcp_ab, rcp_a)
                prev = (h, pts)

            # normalize chunks 0-6 first (rcp was hoisted to head 13, so the
            # psb matmuls start immediately); the DVE mul chain then drains
            # under the last head's att@v matmuls, and every otc chunk is
            # ready before the output projection reads it
            for c in range(7):
                psb = p_st.tile([128, S], f32, tag="st", name=f"psb{c}")
                for off, wdt in ICH:
                    nc.tensor.matmul(
                        psb[:, off : off + wdt],
                        sel16[0:14, c * 128 : (c + 1) * 128],
                        rcp_ab[:, off : off + wdt],
                        start=True,
                        stop=True,
                    )
                    nc.vector.tensor_mul(
                        otc[c][:, off : off + wdt],
                        otc[c][:, off : off + wdt],
                        psb[:, off : off + wdt],
                    )

            ph, ppts = prev
            pot = [
                p_po.tile([128, 512], f32, tag="ot", name=f"ot{ph}_{i}")
                for i in range(2)
            ]
            for j in range(NJT):
                attv_step(ph, ppts, pot, j)
            finish_head(ph, ppts, pot)

            rcp_b = p_sm.tile([2, S], f32, tag="rcpb")
            nc.vector.reciprocal_approx_fast(rcp_b, cs_b)
            rcp_bb = p_sm.tile([2, S], bf, tag="rcpbb")
            nc.vector.tensor_copy(rcp_bb, rcp_b)

            # ---- output projection (same psum scope: no pool barrier).
            # it-tile 0's chunk 0-6 accumulation is emitted BEFORE chunk 7's
            # normalization so the cs_b->rcp_b wait hides behind real work
            # instead of head-blocking the PE queue.
            p_y = p2.enter_context(tc.tile_pool(name="y", bufs=4))

            def outproj_tile(it, yps, crange, start_c, stop_c):
                rows = 128 if it < 8 else TAIL
                for c in crange:
                    for ci in range(2):
                        nc.tensor.matmul(
                            yps[ci][:rows, :],
                            otc[c][:, it * 128 : it * 128 + rows],
                            wo_t[c][:, ci * 512 : (ci + 1) * 512],
                            start=(c == start_c),
                            stop=(c == stop_c),
                        )

            def outproj_drain(it, yps):
                rows = 128 if it < 8 else TAIL
                for ci in range(2):
                    ysb = p_y.tile([128, 512], bf, tag="ysb")
                    nc.scalar.copy(ysb[:rows, :], yps[ci][:rows, :])
                    (nc.sync if ci == 0 else nc.gpsimd).dma_start(
                        out=OUT[0:rows, it * D + ci * 512 : it * D + (ci + 1) * 512],
                        in_=ysb[:rows, :],
                    )

            yps0 = [
                p_st.tile([128, 512], f32, tag="st", name="y0_0"),
                p_po.tile([128, 512], f32, tag="ot", name="y0_1"),
            ]
            outproj_tile(0, yps0, range(7), 0, 7)

            psb7 = p_st.tile([128, S], f32, tag="st", name="psb7")
            for oi, (off, wdt) in enumerate(ICH):
                nc.tensor.matmul(
                    psb7[:, off : off + wdt],
                    sel2b,
                    rcp_bb[:, off : off + wdt],
                    start=True,
                    stop=True,
                )
                nc.vector.tensor_mul(
                    otc[7][:, off : off + wdt],
                    otc[7][:, off : off + wdt],
                    psb7[:, off : off + wdt],
                )

            outproj_tile(0, yps0, [7], 0, 7)
            outproj_drain(0, yps0)
            for it in range(1, NJT):
                yps = [
                    p_st.tile([128, 512], f32, tag="st", name=f"y{it}_0"),
                    p_po.tile([128, 512], f32, tag="ot", name=f"y{it}_1"),
                ]
                outproj_tile(it, yps, range(NCHUNK), 0, 7)
                outproj_drain(it, yps)


def _build():
    global _compiled
    if _compiled is not None:
        return _compiled
    import concourse.bass as bass  # noqa: F401
    import concourse.mybir as mybir
    import concourse.tile as tile
    from concourse import bacc

    nc = bacc.Bacc("TRN2", target_bir_lowering=False, debug=False)
    bf = mybir.dt.bfloat16
    f32 = mybir.dt.float32
    # big inputs host-prepacked chunk-major as [128, chunks*width]
    aps = {
        "xT": nc.dram_tensor("xT", [128, 8 * S], bf, kind="ExternalInput").ap(),
        "Wq": nc.dram_tensor("Wq", [128, 8 * D], bf, kind="ExternalInput").ap(),
        "Wk": nc.dram_tensor("Wk", [128, 8 * D], bf, kind="ExternalInput").ap(),
        "Wv": nc.dram_tensor("Wv", [128, 8 * D], bf, kind="ExternalInput").ap(),
        "Wo": nc.dram_tensor("Wo", [128, 8 * D], bf, kind="ExternalInput").ap(),
        "COS2": nc.dram_tensor("COS2", [128, S], bf, kind="ExternalInput").ap(),
        "S2": nc.dram_tensor("S2", [128, S], bf, kind="ExternalInput").ap(),
        "SWP": nc.dram_tensor("SWP", [128, 128], bf, kind="ExternalInput").ap(),
        "SEL": nc.dram_tensor("SEL", [16, 1024], bf, kind="ExternalInput").ap(),
        "SEL2": nc.dram_tensor("SEL2", [2, 128], bf, kind="ExternalInput").ap(),
        "out": nc.dram_tensor("out", [128, 9 * D], bf, kind="ExternalOutput").ap(),
    }
    with tile.TileContext(nc) as tc:
        _build_body(nc, tc, tile, mybir, aps)
    nc.compile()
    _compiled = nc
    return nc


def _install_trace_shim():
    """The agent image's antenv lacks axon_hooks, so run_bass_kernel_spmd's
    trace path can't find the NTFF profile hook trn_boot would have set.
    Recreate the module and install the ctypes hook; skip the S3 artifact
    upload (no creds needed for local timing)."""
    import sys
    import types

    if "antenv.axon_hooks" not in sys.modules:
        import antenv  # noqa: F401

        mod = types.ModuleType("antenv.axon_hooks")
        mod._hook = None

        def set_axon_ntff_profile_hook(h):
            mod._hook = h

        def get_axon_ntff_profile_hook():
            return mod._hook

        mod.set_axon_ntff_profile_hook = set_axon_ntff_profile_hook
        mod.get_axon_ntff_profile_hook = get_axon_ntff_profile_hook
        sys.modules["antenv.axon_hooks"] = mod

    import antenv.axon_hooks as ah

    if ah.get_axon_ntff_profile_hook() is None:
        from trn_agent_boot.trn_boot import _ntff_profile_via_ctypes

        ah.set_axon_ntff_profile_hook(
            _ntff_profile_via_ctypes("/opt/axon/libaxon_pjrt.so")
        )

    import concourse.bass_utils as bu

    bu.upload_artifacts = lambda tmpdir: f"local://{tmpdir}"


def run(inputs, trace=False):
    """Returns (output (8,1040,1024) f32, exec_time_ns or None)."""
    if trace:
        _install_trace_shim()
    from concourse.bass_utils import run_bass_kernel_spmd

    nc = _build()

    def chunk_major(a):  # [8*128, W] -> [128, 8*W]
        w = a.shape[1]
        return np.ascontiguousarray(
            a.reshape(8, 128, w).transpose(1, 0, 2).reshape(128, 8 * w)
        )

    x = np.asarray(inputs["x"], np.float32)
    wq = chunk_major(np.asarray(inputs["Wq"], np.float32).reshape(D, H * HD)).astype(BF16)
    wk = chunk_major(np.asarray(inputs["Wk"], np.float32).reshape(D, H * HD)).astype(BF16)
    wv = chunk_major(np.asarray(inputs["Wv"], np.float32).reshape(D, H * HD)).astype(BF16)
    wo = chunk_major(np.asarray(inputs["Wo"], np.float32).reshape(H * HD, D)).astype(BF16)
    cos2, s2 = _rope_tables()
    swp = _swap_matrix()
    sel = np.zeros((16, 1024), np.float32)
    for c in range(8):
        for hb in range(2):
            sel[2 * c + hb, c * 128 + hb * 64 : c * 128 + hb * 64 + 64] = 1.0
    sel2b = np.zeros((2, 128), np.float32)
    sel2b[0, 0:64] = 1.0
    sel2b[1, 64:128] = 1.0
    shared = {
        "Wq": wq, "Wk": wk, "Wv": wv, "Wo": wo,
        "COS2": cos2, "S2": s2, "SWP": swp,
        "SEL": sel.astype(BF16), "SEL2": sel2b.astype(BF16),
    }
    in_maps = [
        dict(shared, xT=chunk_major(np.ascontiguousarray(x[b].T)).astype(BF16))
        for b in range(B)
    ]
    res = run_bass_kernel_spmd(nc, in_maps, core_ids=list(range(B)), trace=trace)
    out = np.stack(
        [
            np.asarray(r["out"], np.float32)
            .reshape(128, 9, D)
            .transpose(1, 0, 2)
            .reshape(9 * 128, D)[:S]
            for r in res.results
        ],
        axis=0,
    )
    return out, res.exec_time_ns


def kernel(x, Wq, Wk, Wv, Wo):
    out, _ = run({"x": x, "Wq": Wq, "Wk": Wk, "Wv": Wv, "Wo": Wo})
    return out

